# revision 7
# baseline (speedup 1.0000x reference)
"""Trainium2 Bass kernel for nn_Att_2_layer1 (ragged attention over boxes).

Computation (reference):
  v_proj = relu(v @ Wv.T + bv)            [N,K,H]
  q_proj = relu(q @ Wq.T + bq)            [N,H]
  joint  = v_proj * q_proj[:,None,:]      [N,K,H]
  logits = joint @ Wl[0] + bl             [N,K]
  pad_sequence(tags_attention) gather -> [B,S,T,K]   (identity when tags==1)
  w = masked_softmax(logits_batch, box_mask)

Sharding: data-parallel over the flat tag dim NB (8 cores x 1024 rows),
weights replicated.  Host pre-transposes v and q to [d, nk] bf16 layout
(zero on-device transposes, plain HWDGE loads).  Column order within a
128-n group: j = q4*1152 + k*32 + m  (q4 = n//32 stripe, m = n%32), so
the G-matmul diag extract reduces contiguously.  Per core, per group:
  - DMA vT chunk [128d, 2dh, 4608] bf16 (2.36 MB contiguous),
  - vproj: 9 x 512-col chunks, 2 dh-accumulated matmuls per hh half,
    relu+bias PSUM->SBUF copy on Scalar/Vector -> vp bf16,
  - G-matmul: lhsT = gT 32-n' slices (q_proj.T * Wl), 4 stripes packed
    via tile_position -> PSUM [128, 1152] per group,
  - block-diag extract (mask-mult + contiguous reduce) -> z36 [128, 36],
  - batched masked softmax over all groups at the end, single out DMA.
"""

import os
import numpy as np

B, S, T, K = 128, 4, 16, 36
VD, QD, H = 256, 256, 256
NB = B * S * T              # 8192
NCORES = 8
NPC = NB // NCORES          # 1024 n-rows per core
SBN = 32                    # n-rows per superblock (stripe)
SBK = SBN * K               # 1152 nk per superblock
NG = 8                      # groups of 128 n per core
GK = 128 * K                # 4608 nk per group
FB = 384                    # free-dim block (3 per superblock)
VC = 512                    # vproj chunk width (one PSUM bank)
NVC = GK // VC              # 9 vproj chunks per group

_CACHE = {}


def _build_module():
    import concourse.bass as bass
    import concourse.mybir as mybir
    import concourse.tile as tile
    from concourse import bacc
    from contextlib import ExitStack

    f32 = mybir.dt.float32
    bf16 = mybir.dt.bfloat16

    nc = bacc.Bacc("TRN2", target_bir_lowering=False)

    vt_d = nc.dram_tensor("vt", [NG * 128, 2 * GK], bf16, kind="ExternalInput")
    qt_d = nc.dram_tensor("qt", [128, 2 * NPC], bf16, kind="ExternalInput")
    # packed constants: c16 = wvt(512) | wqt(512); c32 = mdiag | msm | bv bq wl blc
    c16_d = nc.dram_tensor("c16", [128, 2 * 512], bf16, kind="ExternalInput")
    c32_d = nc.dram_tensor("c32", [128, SBK + NG * K + 7], f32,
                           kind="ExternalInput")
    out_d = nc.dram_tensor("out_w", [128, NG * K], f32, kind="ExternalOutput")

    with tile.TileContext(nc) as tc, ExitStack() as ctx:
        singles = ctx.enter_context(tc.tile_pool(name="singles", bufs=1))

        # constants ride the scalar HWDGE queue, concurrent with v loads
        c16 = singles.tile([128, 2 * 512], bf16)
        nc.scalar.dma_start(out=c16, in_=c16_d[:])
        c32 = singles.tile([128, SBK + NG * K + 7], f32)
        nc.scalar.dma_start(out=c32, in_=c32_d[:])
        wvt = c16[:, 0:512].rearrange("p (dh h) -> p dh h", dh=2, h=H)
        wqt = c16[:, 512:1024].rearrange("p (dh h) -> p dh h", dh=2, h=H)
        mdiag = c32[:, 0:SBK]
        msm = c32[:, SBK:SBK + NG * K]
        co = SBK + NG * K
        bv = c32[:, co:co + 2]
        bq = c32[:, co + 2:co + 4]
        wl = c32[:, co + 4:co + 6]
        blc = c32[:, co + 6:co + 7]
        gT = singles.tile([128, 2, NPC], bf16)     # q_proj.T * Wl  [h, n]
        wg = singles.tile([128, NG * K], f32)      # final weights, all groups

        # ---------------- pools (allocated before any DMA ordering) --------
        vin_pool = ctx.enter_context(tc.tile_pool(name="vin", bufs=4))
        vp_pool = ctx.enter_context(tc.tile_pool(name="vp", bufs=2))
        d_pool = ctx.enter_context(tc.tile_pool(name="dsb", bufs=2))
        qpool = ctx.enter_context(tc.tile_pool(name="qpool", bufs=1))
        vp_ps = ctx.enter_context(tc.tile_pool(name="vp_ps", bufs=2, space="PSUM"))
        g_ps = ctx.enter_context(tc.tile_pool(name="g_ps", bufs=1, space="PSUM"))

        # first v chunk in two halves so vproj can start ~4us earlier
        vt0 = vin_pool.tile([128, 2, GK], bf16, tag="vt")
        nc.sync.dma_start(
            out=vt0[:, :, 0:2048],
            in_=bass.AP(vt_d, 0, [[2 * GK, 128], [GK, 2], [1, 2048]]))
        nc.sync.dma_start(
            out=vt0[:, :, 2048:GK],
            in_=bass.AP(vt_d, 2048, [[2 * GK, 128], [GK, 2], [1, GK - 2048]]))

        # ---------------- Q phase: gT = (relu(qT.T Wq + bq)).T * Wl --------
        qT = qpool.tile([128, 2, NPC], bf16, tag="qT")
        nc.scalar.dma_start(
            out=qT,
            in_=bass.AP(qt_d, 0, [[2 * NPC, 128], [NPC, 2], [1, NPC]]))

        for hh in range(2):
            for blk in range(2):  # n blocks of 512
                ps = vp_ps.tile([128, 512], f32, name=f"qmm{hh}{blk}",
                                tag=f"v{hh}")
                for dh in range(2):
                    nc.tensor.matmul(
                        ps,
                        wqt[:, dh, hh * 128:(hh + 1) * 128],
                        qT[:, dh, blk * 512:(blk + 1) * 512],
                        start=(dh == 0), stop=(dh == 1),
                    )
                tmp = qpool.tile([128, 512], f32, tag=f"qrelu{hh}{blk}")
                nc.scalar.activation(
                    out=tmp, in_=ps,
                    func=mybir.ActivationFunctionType.Relu,
                    bias=bq[:, hh:hh + 1], scale=1.0,
                )
                nc.vector.tensor_scalar_mul(
                    gT[:, hh, blk * 512:(blk + 1) * 512],
                    tmp, wl[:, hh:hh + 1])

        # ---------------- main loop over 128-n groups ----------------------
        for g in range(NG):
            if g == 0:
                vtile = vt0
            else:
                vtile = vin_pool.tile([128, 2, GK], bf16, name=f"vt{g}",
                                      tag="vt")
                nc.sync.dma_start(
                    out=vtile,
                    in_=bass.AP(vt_d, g * 128 * 2 * GK,
                                [[2 * GK, 128], [GK, 2], [1, GK]]))
            vp = vp_pool.tile([128, 2, GK], bf16, tag="vp")

            for c in range(NVC):
                for hh in range(2):
                    ps = vp_ps.tile([128, VC], f32, name=f"ps{g}_{c}_{hh}",
                                    tag=f"v{hh}")
                    for dh in range(2):
                        nc.tensor.matmul(
                            ps,
                            wvt[:, dh, hh * 128:(hh + 1) * 128],
                            vtile[:, dh, c * VC:(c + 1) * VC],
                            start=(dh == 0), stop=(dh == 1),
                        )
                    dst = vp[:, hh, c * VC:(c + 1) * VC]
                    if (c * 2 + hh) % 3 != 0:   # 12 on Scalar, 6 on Vector
                        nc.scalar.activation(
                            out=dst, in_=ps,
                            func=mybir.ActivationFunctionType.Relu,
                            bias=bv[:, hh:hh + 1], scale=1.0,
                        )
                    else:
                        nc.vector.tensor_scalar(
                            out=dst, in0=ps,
                            scalar1=bv[:, hh:hh + 1], scalar2=0.0,
                            op0=mybir.AluOpType.add, op1=mybir.AluOpType.max,
                        )

            # G-matmul: 4 stripes of 32 n' packed via tile_position
            dsb = d_pool.tile([128, SBK], f32, tag="dsb")
            for blk in range(3):
                gt = g_ps.tile([128, FB], f32, name=f"gt{g}_{blk}",
                               tag=f"g{blk}")
                for hh in range(2):
                    for q4 in range(4):
                        stripe = 32 * q4
                        nc.tensor.matmul(
                            gt[stripe:stripe + SBN, :],
                            gT[:, hh, g * 128 + stripe:g * 128 + stripe + SBN],
                            vp[:, hh, q4 * SBK + blk * FB:q4 * SBK + (blk + 1) * FB],
                            start=(hh == 0), stop=(hh == 1),
                            tile_position=(0, stripe),
                            skip_group_check=True,
                        )
                nc.vector.tensor_mul(
                    dsb[:, blk * FB:(blk + 1) * FB],
                    gt, mdiag[:, blk * FB:(blk + 1) * FB],
                )
            # contiguous diag reduce: z36[p, k] = sum_m dsb[p, k*32 + m]
            z36 = d_pool.tile([128, K], f32, name=f"z36_{g}", tag="z36")
            nc.vector.tensor_reduce(
                out=z36,
                in_=dsb.rearrange("p (k m) -> p k m", k=K, m=SBN),
                axis=mybir.AxisListType.X,
                op=mybir.AluOpType.add,
            )
            # ---- masked softmax for this group (reference semantics;
            # |logits| < 3 so max-subtraction is unnecessary in f32) ----
            msl = msm[:, g * K:(g + 1) * K]
            nc.vector.tensor_scalar_add(z36, z36, blc)
            nc.vector.tensor_mul(z36, z36, msl)
            e36 = d_pool.tile([128, K], f32, name=f"e36_{g}", tag="e36")
            nc.scalar.activation(out=e36, in_=z36,
                                 func=mybir.ActivationFunctionType.Exp)
            sall = d_pool.tile([128, 1], f32, name=f"sall_{g}", tag="sall")
            nc.vector.tensor_reduce(out=sall, in_=e36,
                                    axis=mybir.AxisListType.X,
                                    op=mybir.AluOpType.add)
            e2 = d_pool.tile([128, K], f32, name=f"e2_{g}", tag="e2")
            nc.vector.tensor_mul(e2, e36, msl)
            s2 = d_pool.tile([128, 1], f32, name=f"s2_{g}", tag="s2")
            nc.vector.tensor_reduce(out=s2, in_=e2,
                                    axis=mybir.AxisListType.X,
                                    op=mybir.AluOpType.add)
            nc.vector.tensor_scalar_mul(sall, sall, 1e-13)
            denom = d_pool.tile([128, 1], f32, name=f"dn_{g}", tag="dn")
            nc.vector.tensor_add(denom, s2, sall)
            rec = d_pool.tile([128, 1], f32, name=f"rec_{g}", tag="rec")
            nc.vector.reciprocal(out=rec, in_=denom)
            nc.vector.tensor_scalar_mul(
                wg[:, g * K:(g + 1) * K], e2, rec)

        nc.scalar.dma_start(out=out_d[:], in_=wg)

    nc.finalize()
    return nc


def _host_prep(v, q, box_mask, Wv, bv, Wq, bq, Wl, bl):
    import ml_dtypes
    bf16 = ml_dtypes.bfloat16

    # vT [c, g, p, dh, j] with j = q4*1152 + k*32 + m, d = dh*128 + p
    vt = v.reshape(NCORES, NG, 4, SBN, K, VD).astype(bf16)
    vt = vt.transpose(0, 1, 5, 2, 4, 3)          # [c, g, d, q4, k, m]
    vt = vt.reshape(NCORES, NG, 2, 128, GK)
    vt = np.ascontiguousarray(vt.transpose(0, 1, 3, 2, 4))  # [c, g, p, dh, j]
    vt = vt.reshape(NCORES, NG * 128, 2 * GK)

    qt = q.reshape(NCORES, NPC, QD).astype(bf16)
    qt = qt.transpose(0, 2, 1).reshape(NCORES, 2, 128, NPC)
    qt = np.ascontiguousarray(qt.transpose(0, 2, 1, 3))     # [c, p, dh, n]
    qt = qt.reshape(NCORES, 128, 2 * NPC)

    # wvt[p, dh, h] = Wv[h, dh*128+p]
    wvt = Wv.T.reshape(2, 128, H).transpose(1, 0, 2).reshape(128, 512)
    wqt = Wq.T.reshape(2, 128, H).transpose(1, 0, 2).reshape(128, 512)
    c16 = np.ascontiguousarray(np.concatenate([wvt, wqt], axis=1)).astype(bf16)
    # mdiag[p, k*32 + m] = 1 iff m == p % 32
    mdiag = np.zeros((128, SBK), dtype=np.float32)
    for p in range(128):
        mdiag[p, (p % SBN)::SBN] = 1.0

    in_maps = []
    for c in range(NCORES):
        n0 = c * NPC
        # msm[p, g*K + k] = box_mask[b(n)] with global n = n0 + g*128 + p
        nloc = (np.arange(NG)[None, :] * 128 + np.arange(128)[:, None])
        bidx = (n0 + nloc) // (S * T)          # [128, NG]
        msm = box_mask[bidx]                   # [128, NG, K]
        msm = msm.reshape(128, NG * K).astype(np.float32)
        small = np.stack([
            bv[:128], bv[128:], bq[:128], bq[128:],
            Wl[0, :128], Wl[0, 128:], np.full(128, bl[0], np.float32),
        ], axis=1).astype(np.float32)
        c32 = np.ascontiguousarray(
            np.concatenate([mdiag, msm, small], axis=1))
        in_maps.append(dict(vt=vt[c], qt=qt[c], c16=c16, c32=c32))
    return in_maps


def _numpy_fallback(v, q, box_mask, tags_attention, Wv, bv, Wq, bq, Wl, bl):
    v_proj = np.maximum(v @ Wv.T + bv, 0.0)
    q_proj = np.maximum(q @ Wq.T + bq, 0.0)
    logits = (v_proj * q_proj[:, None, :]) @ Wl[0] + bl[0]
    lengths = tags_attention.sum(-1)
    flat_len = lengths.reshape(-1)
    offsets = np.concatenate([[0], np.cumsum(flat_len)[:-1]]).reshape(B, S)
    t = np.arange(T)
    idx = offsets[:, :, None] + t
    valid = t[None, None, :] < lengths[:, :, None]
    gathered = logits[np.clip(idx, 0, logits.shape[0] - 1)]
    lb = np.where(valid[..., None], gathered, 0.0)
    mask = box_mask[:, None, None, :]
    zz = lb * mask
    zz = zz - zz.max(-1, keepdims=True)
    ee = np.exp(zz)
    sm = ee / ee.sum(-1, keepdims=True)
    w = sm * mask
    w = w / (w.sum(-1, keepdims=True) + 1e-13)
    return w.astype(np.float32)


def kernel(v, q, box_mask, tags_attention, Wv, bv, Wq, bq, Wl, bl):
    v = np.asarray(v, dtype=np.float32)
    q = np.asarray(q, dtype=np.float32)
    box_mask = np.asarray(box_mask, dtype=np.float32)
    tags = np.asarray(tags_attention)
    Wv = np.asarray(Wv, dtype=np.float32); bv = np.asarray(bv, dtype=np.float32)
    Wq = np.asarray(Wq, dtype=np.float32); bq = np.asarray(bq, dtype=np.float32)
    Wl = np.asarray(Wl, dtype=np.float32); bl = np.asarray(bl, dtype=np.float32)

    if not np.all(tags == 1):
        return _numpy_fallback(v, q, box_mask, tags, Wv, bv, Wq, bq, Wl, bl)

    from concourse.bass_utils import run_bass_kernel_spmd

    if "nc" not in _CACHE:
        _CACHE["nc"] = _build_module()
    nc = _CACHE["nc"]

    in_maps = _host_prep(v, q, box_mask, Wv, bv, Wq, bq, Wl, bl)
    res = run_bass_kernel_spmd(
        nc, in_maps, core_ids=list(range(NCORES)),
        trace=bool(int(os.environ.get("BASS_KERNEL_TRACE", "0"))),
    )
    _CACHE["last_results"] = res
    # out_w[p, g*K + k] is the row n = g*128 + p of this core's shard
    w = np.concatenate(
        [r["out_w"].reshape(128, NG, K).transpose(1, 0, 2).reshape(NPC, K)
         for r in res.results], axis=0)
    return np.ascontiguousarray(w.reshape(B, S, T, K))


# revision 9
# speedup vs baseline: 1.1566x; 1.1566x over previous
"""Trainium2 Bass kernel for nn_Att_2_layer1 (ragged attention over boxes).

Computation (reference):
  v_proj = relu(v @ Wv.T + bv)            [N,K,H]
  q_proj = relu(q @ Wq.T + bq)            [N,H]
  joint  = v_proj * q_proj[:,None,:]      [N,K,H]
  logits = joint @ Wl[0] + bl             [N,K]
  pad_sequence(tags_attention) gather -> [B,S,T,K]   (identity when tags==1)
  w = masked_softmax(logits_batch, box_mask)

Sharding: data-parallel over the flat tag dim NB (8 cores x 1024 rows),
weights replicated.  Host pre-transposes v and q to [d, nk] bf16 layout
(zero on-device transposes, plain HWDGE loads).  Column order within a
128-n group: j = q4*1152 + k*32 + m  (q4 = n//32 stripe, m = n%32), so
the G-matmul diag extract reduces contiguously.  Per core, per group:
  - DMA vT chunk [128d, 2dh, 4608] bf16 (2.36 MB contiguous),
  - vproj: 9 x 512-col chunks, 2 dh-accumulated matmuls per hh half,
    relu+bias PSUM->SBUF copy on Scalar/Vector -> vp bf16,
  - G-matmul: lhsT = gT 32-n' slices (q_proj.T * Wl), 4 stripes packed
    via tile_position -> PSUM [128, 1152] per group,
  - block-diag extract (mask-mult + contiguous reduce) -> z36 [128, 36],
  - batched masked softmax over all groups at the end, single out DMA.
"""

import os
import numpy as np

B, S, T, K = 128, 4, 16, 36
VD, QD, H = 256, 256, 256
NB = B * S * T              # 8192
NCORES = 8
NPC = NB // NCORES          # 1024 n-rows per core
SBN = 32                    # n-rows per superblock (stripe)
SBK = SBN * K               # 1152 nk per superblock
NG = 8                      # groups of 128 n per core
GK = 128 * K                # 4608 nk per group
FB = 384                    # free-dim block (3 per superblock)
VC = 512                    # vproj chunk width (one PSUM bank)
NVC = GK // VC              # 9 vproj chunks per group

_CACHE = {}


def _build_module():
    import concourse.bass as bass
    import concourse.mybir as mybir
    import concourse.tile as tile
    from concourse import bacc
    from contextlib import ExitStack

    f32 = mybir.dt.float32
    bf16 = mybir.dt.bfloat16

    nc = bacc.Bacc("TRN2", target_bir_lowering=False)

    vt_d = nc.dram_tensor("vt", [NG * 128, 2 * GK], bf16, kind="ExternalInput")
    qt_d = nc.dram_tensor("qt", [128, 2 * NPC], bf16, kind="ExternalInput")
    # packed constants: c16 = wvt(512) | wqt(512); c32 = mdiag | msm | bv bq wl blc
    c16_d = nc.dram_tensor("c16", [128, 2 * 512], bf16, kind="ExternalInput")
    c32_d = nc.dram_tensor("c32", [128, SBK + NG * K + 7], f32,
                           kind="ExternalInput")
    out_d = nc.dram_tensor("out_w", [128, NG * K], f32, kind="ExternalOutput")

    with tile.TileContext(nc) as tc, ExitStack() as ctx:
        singles = ctx.enter_context(tc.tile_pool(name="singles", bufs=1))

        # c16 (matmul weights) first on the sync queue; the rest of the
        # constants ride the scalar HWDGE queue, concurrent with v loads
        c16 = singles.tile([128, 2 * 512], bf16)
        nc.sync.dma_start(out=c16, in_=c16_d[:])
        c32 = singles.tile([128, SBK + NG * K + 7], f32)
        nc.scalar.dma_start(out=c32, in_=c32_d[:])
        wvt = c16[:, 0:512].rearrange("p (dh h) -> p dh h", dh=2, h=H)
        wqt = c16[:, 512:1024].rearrange("p (dh h) -> p dh h", dh=2, h=H)
        mdiag = c32[:, 0:SBK]
        msm = c32[:, SBK:SBK + NG * K]
        co = SBK + NG * K
        bv = c32[:, co:co + 2]
        bq = c32[:, co + 2:co + 4]
        wl = c32[:, co + 4:co + 6]
        blc = c32[:, co + 6:co + 7]
        gT = singles.tile([128, 2, NPC], bf16)     # q_proj.T * Wl  [h, n]
        wg = singles.tile([128, NG * K], f32)      # final weights, all groups

        # ---------------- pools (allocated before any DMA ordering) --------
        vin_pool = ctx.enter_context(tc.tile_pool(name="vin", bufs=5))
        vp_pool = ctx.enter_context(tc.tile_pool(name="vp", bufs=2))
        d_pool = ctx.enter_context(tc.tile_pool(name="dsb", bufs=2))
        qpool = ctx.enter_context(tc.tile_pool(name="qpool", bufs=1))
        vp_ps = ctx.enter_context(tc.tile_pool(name="vp_ps", bufs=2, space="PSUM"))
        g_ps = ctx.enter_context(tc.tile_pool(name="g_ps", bufs=1, space="PSUM"))

        # first v chunk in three pieces so vproj can start ~7us earlier
        vt0 = vin_pool.tile([128, 2, GK], bf16, tag="vt")
        for lo, hi in ((0, 512), (512, 1536), (1536, GK)):
            nc.sync.dma_start(
                out=vt0[:, :, lo:hi],
                in_=bass.AP(vt_d, lo, [[2 * GK, 128], [GK, 2], [1, hi - lo]]))

        qT = qpool.tile([128, 2, NPC], bf16, tag="qT")
        nc.scalar.dma_start(
            out=qT,
            in_=bass.AP(qt_d, 0, [[2 * NPC, 128], [NPC, 2], [1, NPC]]))

        def emit_q_phase():
            for hh in range(2):
                for blk in range(2):  # n blocks of 512
                    ps = vp_ps.tile([128, 512], f32, name=f"qmm{hh}{blk}",
                                    tag=f"v{hh}")
                    for dh in range(2):
                        nc.tensor.matmul(
                            ps,
                            wqt[:, dh, hh * 128:(hh + 1) * 128],
                            qT[:, dh, blk * 512:(blk + 1) * 512],
                            start=(dh == 0), stop=(dh == 1),
                        )
                    tmp = qpool.tile([128, 512], f32, tag=f"qrelu{hh}{blk}")
                    nc.scalar.activation(
                        out=tmp, in_=ps,
                        func=mybir.ActivationFunctionType.Relu,
                        bias=bq[:, hh:hh + 1], scale=1.0,
                    )
                    nc.vector.tensor_scalar_mul(
                        gT[:, hh, blk * 512:(blk + 1) * 512],
                        tmp, wl[:, hh:hh + 1])

        vps = {}

        def emit_vproj(g, vtile, c0, c1):
            vp = vps[g]
            for c in range(c0, c1):
                for hh in range(2):
                    ps = vp_ps.tile([128, VC], f32, name=f"ps{g}_{c}_{hh}",
                                    tag=f"v{hh}")
                    for dh in range(2):
                        nc.tensor.matmul(
                            ps,
                            wvt[:, dh, hh * 128:(hh + 1) * 128],
                            vtile[:, dh, c * VC:(c + 1) * VC],
                            start=(dh == 0), stop=(dh == 1),
                        )
                    dst = vp[:, hh, c * VC:(c + 1) * VC]
                    if (c * 2 + hh) % 3 != 0:   # 12 on Scalar, 6 on Vector
                        nc.scalar.activation(
                            out=dst, in_=ps,
                            func=mybir.ActivationFunctionType.Relu,
                            bias=bv[:, hh:hh + 1], scale=1.0,
                        )
                    else:
                        nc.vector.tensor_scalar(
                            out=dst, in0=ps,
                            scalar1=bv[:, hh:hh + 1], scalar2=0.0,
                            op0=mybir.AluOpType.add, op1=mybir.AluOpType.max,
                        )

        def emit_g_phase(g):
            vp = vps.pop(g)
            # G-matmul: 4 stripes of 32 n' packed via tile_position
            dsb = d_pool.tile([128, SBK], f32, name=f"dsb{g}", tag="dsb")
            for blk in range(3):
                gt = g_ps.tile([128, FB], f32, name=f"gt{g}_{blk}",
                               tag=f"g{blk}")
                for hh in range(2):
                    for q4 in range(4):
                        stripe = 32 * q4
                        nc.tensor.matmul(
                            gt[stripe:stripe + SBN, :],
                            gT[:, hh, g * 128 + stripe:g * 128 + stripe + SBN],
                            vp[:, hh, q4 * SBK + blk * FB:q4 * SBK + (blk + 1) * FB],
                            start=(hh == 0), stop=(hh == 1),
                            tile_position=(0, stripe),
                            skip_group_check=True,
                        )
                nc.vector.tensor_mul(
                    dsb[:, blk * FB:(blk + 1) * FB],
                    gt, mdiag[:, blk * FB:(blk + 1) * FB],
                )
                # partial diag reduce: z36[p, k] = sum_m dsb[p, k*32 + m]
                if blk == 0:
                    z36 = d_pool.tile([128, K], f32, name=f"z36_{g}",
                                      tag="z36")
                nc.vector.tensor_reduce(
                    out=z36[:, blk * 12:(blk + 1) * 12],
                    in_=dsb[:, blk * FB:(blk + 1) * FB].rearrange(
                        "p (k m) -> p k m", k=12, m=SBN),
                    axis=mybir.AxisListType.X,
                    op=mybir.AluOpType.add,
                )
            # ---- masked softmax for this group (reference semantics;
            # |logits| < 3 so max-subtraction is unnecessary in f32) ----
            msl = msm[:, g * K:(g + 1) * K]
            nc.vector.tensor_scalar_add(z36, z36, blc)
            nc.vector.tensor_mul(z36, z36, msl)
            e36 = d_pool.tile([128, K], f32, name=f"e36_{g}", tag="e36")
            nc.scalar.activation(out=e36, in_=z36,
                                 func=mybir.ActivationFunctionType.Exp)
            sall = d_pool.tile([128, 1], f32, name=f"sall_{g}", tag="sall")
            nc.vector.tensor_reduce(out=sall, in_=e36,
                                    axis=mybir.AxisListType.X,
                                    op=mybir.AluOpType.add)
            e2 = d_pool.tile([128, K], f32, name=f"e2_{g}", tag="e2")
            nc.vector.tensor_mul(e2, e36, msl)
            s2 = d_pool.tile([128, 1], f32, name=f"s2_{g}", tag="s2")
            nc.vector.tensor_reduce(out=s2, in_=e2,
                                    axis=mybir.AxisListType.X,
                                    op=mybir.AluOpType.add)
            nc.vector.tensor_scalar_mul(sall, sall, 1e-13)
            denom = d_pool.tile([128, 1], f32, name=f"dn_{g}", tag="dn")
            nc.vector.tensor_add(denom, s2, sall)
            rec = d_pool.tile([128, 1], f32, name=f"rec_{g}", tag="rec")
            nc.vector.reciprocal(out=rec, in_=denom)
            nc.vector.tensor_scalar_mul(
                wg[:, g * K:(g + 1) * K], e2, rec)

        # ---------------- software-pipelined main loop ---------------------
        # Group g's G phase is emitted inside group g+1's vproj so its rhs
        # (vp of g) is fully relu'd by then -> zero PE stalls on G.
        for g in range(NG):
            if g == 0:
                vtile = vt0
            else:
                vtile = vin_pool.tile([128, 2, GK], bf16, name=f"vt{g}",
                                      tag="vt")
                nc.sync.dma_start(
                    out=vtile,
                    in_=bass.AP(vt_d, g * 128 * 2 * GK,
                                [[2 * GK, 128], [GK, 2], [1, GK]]))
            vps[g] = vp_pool.tile([128, 2, GK], bf16, name=f"vp{g}", tag="vp")
            emit_vproj(g, vtile, 0, 3)
            if g == 0:
                emit_q_phase()
            else:
                emit_g_phase(g - 1)
            emit_vproj(g, vtile, 3, NVC)
        emit_g_phase(NG - 1)

        nc.scalar.dma_start(out=out_d[:], in_=wg)

    nc.finalize()
    return nc


def _host_prep(v, q, box_mask, Wv, bv, Wq, bq, Wl, bl):
    import ml_dtypes
    bf16 = ml_dtypes.bfloat16

    # vT [c, g, p, dh, j] with j = q4*1152 + k*32 + m, d = dh*128 + p
    vt = v.reshape(NCORES, NG, 4, SBN, K, VD).astype(bf16)
    vt = vt.transpose(0, 1, 5, 2, 4, 3)          # [c, g, d, q4, k, m]
    vt = vt.reshape(NCORES, NG, 2, 128, GK)
    vt = np.ascontiguousarray(vt.transpose(0, 1, 3, 2, 4))  # [c, g, p, dh, j]
    vt = vt.reshape(NCORES, NG * 128, 2 * GK)

    qt = q.reshape(NCORES, NPC, QD).astype(bf16)
    qt = qt.transpose(0, 2, 1).reshape(NCORES, 2, 128, NPC)
    qt = np.ascontiguousarray(qt.transpose(0, 2, 1, 3))     # [c, p, dh, n]
    qt = qt.reshape(NCORES, 128, 2 * NPC)

    # wvt[p, dh, h] = Wv[h, dh*128+p]
    wvt = Wv.T.reshape(2, 128, H).transpose(1, 0, 2).reshape(128, 512)
    wqt = Wq.T.reshape(2, 128, H).transpose(1, 0, 2).reshape(128, 512)
    c16 = np.ascontiguousarray(np.concatenate([wvt, wqt], axis=1)).astype(bf16)
    # mdiag[p, k*32 + m] = 1 iff m == p % 32
    mdiag = np.zeros((128, SBK), dtype=np.float32)
    for p in range(128):
        mdiag[p, (p % SBN)::SBN] = 1.0

    in_maps = []
    for c in range(NCORES):
        n0 = c * NPC
        # msm[p, g*K + k] = box_mask[b(n)] with global n = n0 + g*128 + p
        nloc = (np.arange(NG)[None, :] * 128 + np.arange(128)[:, None])
        bidx = (n0 + nloc) // (S * T)          # [128, NG]
        msm = box_mask[bidx]                   # [128, NG, K]
        msm = msm.reshape(128, NG * K).astype(np.float32)
        small = np.stack([
            bv[:128], bv[128:], bq[:128], bq[128:],
            Wl[0, :128], Wl[0, 128:], np.full(128, bl[0], np.float32),
        ], axis=1).astype(np.float32)
        c32 = np.ascontiguousarray(
            np.concatenate([mdiag, msm, small], axis=1))
        in_maps.append(dict(vt=vt[c], qt=qt[c], c16=c16, c32=c32))
    return in_maps


def _numpy_fallback(v, q, box_mask, tags_attention, Wv, bv, Wq, bq, Wl, bl):
    v_proj = np.maximum(v @ Wv.T + bv, 0.0)
    q_proj = np.maximum(q @ Wq.T + bq, 0.0)
    logits = (v_proj * q_proj[:, None, :]) @ Wl[0] + bl[0]
    lengths = tags_attention.sum(-1)
    flat_len = lengths.reshape(-1)
    offsets = np.concatenate([[0], np.cumsum(flat_len)[:-1]]).reshape(B, S)
    t = np.arange(T)
    idx = offsets[:, :, None] + t
    valid = t[None, None, :] < lengths[:, :, None]
    gathered = logits[np.clip(idx, 0, logits.shape[0] - 1)]
    lb = np.where(valid[..., None], gathered, 0.0)
    mask = box_mask[:, None, None, :]
    zz = lb * mask
    zz = zz - zz.max(-1, keepdims=True)
    ee = np.exp(zz)
    sm = ee / ee.sum(-1, keepdims=True)
    w = sm * mask
    w = w / (w.sum(-1, keepdims=True) + 1e-13)
    return w.astype(np.float32)


def kernel(v, q, box_mask, tags_attention, Wv, bv, Wq, bq, Wl, bl):
    v = np.asarray(v, dtype=np.float32)
    q = np.asarray(q, dtype=np.float32)
    box_mask = np.asarray(box_mask, dtype=np.float32)
    tags = np.asarray(tags_attention)
    Wv = np.asarray(Wv, dtype=np.float32); bv = np.asarray(bv, dtype=np.float32)
    Wq = np.asarray(Wq, dtype=np.float32); bq = np.asarray(bq, dtype=np.float32)
    Wl = np.asarray(Wl, dtype=np.float32); bl = np.asarray(bl, dtype=np.float32)

    if not np.all(tags == 1):
        return _numpy_fallback(v, q, box_mask, tags, Wv, bv, Wq, bq, Wl, bl)

    from concourse.bass_utils import run_bass_kernel_spmd

    if "nc" not in _CACHE:
        _CACHE["nc"] = _build_module()
    nc = _CACHE["nc"]

    in_maps = _host_prep(v, q, box_mask, Wv, bv, Wq, bq, Wl, bl)
    res = run_bass_kernel_spmd(
        nc, in_maps, core_ids=list(range(NCORES)),
        trace=bool(int(os.environ.get("BASS_KERNEL_TRACE", "0"))),
    )
    _CACHE["last_results"] = res
    # out_w[p, g*K + k] is the row n = g*128 + p of this core's shard
    w = np.concatenate(
        [r["out_w"].reshape(128, NG, K).transpose(1, 0, 2).reshape(NPC, K)
         for r in res.results], axis=0)
    return np.ascontiguousarray(w.reshape(B, S, T, K))
